# revision 6
# baseline (speedup 1.0000x reference)
"""Trainium2 Bass kernel for CSAFR probe+mask module.

Computation (per batch element n):
    pooled[n,c]  = mean_hw feat[n,c,:,:]
    pred[n,:]    = pooled[n] @ w.T + b
    g[n,c]       = w[y_n, c] * pooled[n,c]
    mask[n,:]    = softmax(1 + g[n]/||g[n]|| * sqrt(C)/2)
    masked[n]    = feat[n] * mask[n,c]

Sharding: data-parallel over batch N=128 -> 16 images per core x 8 cores.
w/b replicated. No collectives.
"""

import math

import numpy as np

# Problem constants (hardcoded per harness contract).
N, C, H, W = 128, 512, 28, 28
HW = H * W                       # 784
NCLS = 1000
NCORES = 8
NPC = N // NCORES                # 16 images per core
GRP = 8                          # images per half-batch
P = 128
NCH = C // P                     # 4 channel chunks
KCH = (NCLS + P - 1) // P        # 8 class chunks (last partial: 104 rows)
LASTK = NCLS - (KCH - 1) * P     # 104


def _split_sync_waits(nc, max_waits=1):
    """This walrus build's codegen rejects instructions carrying >max_waits sem
    waits. Hoist the overflow onto preceding same-engine NoOp carriers."""
    import concourse.mybir as mybir

    uid = [0]
    for fn in nc.m.functions:
        for bb in fn.blocks:
            insts = bb.instructions
            i = 0
            while i < len(insts):
                inst = insts[i]
                si = inst.sync_info
                if si is not None and si.on_wait and len(si.on_wait) > max_waits:
                    waits = list(si.on_wait)
                    extra = [
                        waits[j : j + max_waits]
                        for j in range(max_waits, len(waits), max_waits)
                    ]
                    si.on_wait = waits[:max_waits]
                    for chunk in extra:
                        uid[0] += 1
                        nd = mybir.InstNoOp(
                            name=f"SWsplit-{uid[0]}",
                            engine=inst.engine,
                            bass_nofuse=True,
                            sync_info=mybir.SyncInfo(on_wait=chunk, on_update=[]),
                        )
                        nc.register_instruction(nd)
                        insts.insert(i, nd)
                        i += 1
                i += 1
    return nc


def build_bass(repeat=1):
    """Build the per-core Bass module. `repeat` re-runs the body (same data,
    same result) for overhead-cancelling wall-clock timing in test harnesses."""
    import concourse.bass as bass
    import concourse.tile as tile
    from concourse import mybir

    f32 = mybir.dt.float32
    nc = bass.Bass()

    feat = nc.dram_tensor("feat", [NPC, C, HW], f32, kind="ExternalInput")
    yrow = nc.dram_tensor("yrow", [1, NPC], f32, kind="ExternalInput")
    w_in = nc.dram_tensor("w", [NCLS, C], f32, kind="ExternalInput")
    brow = nc.dram_tensor("brow", [1, NCLS], f32, kind="ExternalInput")  # pre-scaled by HW
    ident = nc.dram_tensor("ident", [P, P], f32, kind="ExternalInput")
    iota8 = nc.dram_tensor("iota8", [P, KCH], f32, kind="ExternalInput")
    masked = nc.dram_tensor("masked", [NPC, C, HW], f32, kind="ExternalOutput")
    pred = nc.dram_tensor("pred", [NPC, NCLS], f32, kind="ExternalOutput")

    # Per-image view: partition p <- channel j*128+p, free (j, hw).
    feat_r = feat[:, :, :].rearrange("n (j p) x -> n p j x", p=P)
    masked_r = masked[:, :, :].rearrange("n (j p) x -> n p j x", p=P)

    X = mybir.AxisListType.X
    EQ = mybir.AluOpType.is_equal
    MUL = mybir.AluOpType.mult

    with tile.TileContext(nc) as tc:
        with (
            tc.tile_pool(name="consts", bufs=1) as consts,
            tc.tile_pool(name="halfp", bufs=2) as halfp,
            tc.tile_pool(name="imgs", bufs=10) as imgs,
            tc.tile_pool(name="small", bufs=2) as small,
            tc.tile_pool(name="psA", bufs=3, space="PSUM") as psA,
            tc.tile_pool(name="psB", bufs=1, space="PSUM") as psB,
        ):
            # ---- constants (loaded once) ----
            ident_sb = consts.tile([P, P], f32)
            nc.sync.dma_start(out=ident_sb, in_=ident[:, :])
            iota_sb = consts.tile([P, KCH], f32)
            nc.sync.dma_start(out=iota_sb, in_=iota8[:, :])
            b_sb = consts.tile([1, NCLS], f32)
            nc.sync.dma_start(out=b_sb, in_=brow[:, :])
            ones_row = consts.tile([1, NPC], f32)
            nc.vector.memset(ones_row, 1.0)
            ones_col = consts.tile([P, 1], f32)
            nc.vector.memset(ones_col, 1.0)

            # y broadcast down all partitions (partition-stride-0 DMA).
            y_bc = consts.tile([P, NPC], f32)
            y_b_ap = bass.AP(tensor=yrow[:, :].tensor, offset=0, ap=[[0, P], [1, NPC]])
            nc.gpsimd.dma_start(out=y_bc, in_=y_b_ap)

            # w in [cls_chunk-on-partition] layout: w_sb[q, k*C+c] = w[k*128+q, c]
            w_sb = consts.tile([P, KCH * C], f32)
            w_head = w_in[: (KCH - 1) * P, :].rearrange("(k q) c -> q k c", q=P)
            nc.sync.dma_start(out=w_sb[:, : (KCH - 1) * C].rearrange("q (k c) -> q k c", k=KCH - 1), in_=w_head)
            # zero the whole tail chunk first (engine-op partition starts must be
            # 0/32/64/96), then overwrite rows 0..LASTK-1 with the real data
            nc.vector.memset(w_sb[:, (KCH - 1) * C :], 0.0)
            nc.sync.dma_start(out=w_sb[:LASTK, (KCH - 1) * C :], in_=w_in[(KCH - 1) * P :, :])

            # w transposed to [c-on-partition, cls] via PE: w_t[p, j, cls] = w[cls, j*128+p]
            w_t = consts.tile([P, NCH, NCLS], f32)
            for k in range(KCH):
                ncls_k = P if k < KCH - 1 else LASTK
                for j in range(NCH):
                    tp = psA.tile([P, P], f32, tag="wt")
                    nc.tensor.transpose(tp, w_sb[:, k * C + j * P : k * C + (j + 1) * P], ident_sb)
                    nc.scalar.copy(out=w_t[:, j, k * P : k * P + ncls_k], in_=tp[:, :ncls_k])

            # one-hot of y in cls-chunk layout: oh[p, k, n] = (y_n == k*128+p)
            oh = consts.tile([P, KCH, NPC], f32)
            for k in range(KCH):
                nc.vector.tensor_scalar(
                    out=oh[:, k, :], in0=y_bc, scalar1=iota_sb[:, k : k + 1],
                    scalar2=None, op0=EQ,
                )

            for _rep in range(repeat):
                for h in range(2):  # half-batches of GRP images
                    n0 = h * GRP
                    # pooled sums in c-layout: pooled[p, j, i] = sum_hw feat[n0+i, j*128+p, :]
                    pooled = halfp.tile([P, NCH, GRP], f32, tag="pooled")
                    img_tiles = []
                    for i in range(GRP):
                        t = imgs.tile([P, NCH, HW], f32, tag="img")
                        nc.sync.dma_start(out=t, in_=feat_r[n0 + i])
                        nc.vector.reduce_sum(out=pooled[:, :, i], in_=t, axis=X)
                        img_tiles.append(t)

                    # pooled -> n-layout [GRP, C] (PE transpose, then copy off PSUM)
                    pnc_ps = psB.tile([GRP, C], f32, tag="pooledt")
                    for j in range(NCH):
                        nc.tensor.transpose(pnc_ps[:, j * P : (j + 1) * P], pooled[:, j, :], ident_sb)
                    pnc = small.tile([GRP, C], f32, tag="pnc")
                    nc.scalar.copy(out=pnc, in_=pnc_ps)

                    # w[y] gather as one-hot matmul: wsel[i, c] = sum_cls oh[cls, i] * w[cls, c]
                    ws_ps = psB.tile([GRP, C], f32, tag="wsel")
                    for k in range(KCH):
                        nc.tensor.matmul(
                            ws_ps, lhsT=oh[:, k, n0 : n0 + GRP],
                            rhs=w_sb[:, k * C : (k + 1) * C],
                            start=(k == 0), stop=(k == KCH - 1),
                        )

                    # g = wsel * pooled_sums  (scale-invariant for the mask: g/||g||)
                    g = small.tile([GRP, C], f32, tag="g")
                    nc.vector.tensor_mul(g, ws_ps, pnc)

                    # ||g||: sum of squares via ACT accumulate, sqrt, reciprocal
                    gsq = small.tile([GRP, C], f32, tag="gsq")
                    ssq = small.tile([GRP, 1], f32, tag="ssq")
                    nc.scalar.activation(out=gsq, in_=g, func=mybir.ActivationFunctionType.Square, accum_out=ssq)
                    nrm = small.tile([GRP, 1], f32, tag="nrm")
                    nc.scalar.sqrt(nrm, ssq)
                    rnrm = small.tile([GRP, 1], f32, tag="rnrm")
                    nc.vector.reciprocal(rnrm, nrm)
                    scl = small.tile([GRP, 1], f32, tag="scl")
                    nc.scalar.mul(scl, rnrm, math.sqrt(C) / 2.0)

                    # softmax without max-subtraction (args bounded in 1 +- sqrt(C)/2):
                    # expm = exp(g*scl + 1), sume = row sum (fused accum)
                    expm = small.tile([GRP, C], f32, tag="expm")
                    sume = small.tile([GRP, 1], f32, tag="sume")
                    nc.scalar.activation(
                        out=expm, in_=g, func=mybir.ActivationFunctionType.Exp,
                        bias=ones_col[:GRP], scale=scl, accum_out=sume,
                    )
                    rsum = small.tile([GRP, 1], f32, tag="rsum")
                    nc.vector.reciprocal(rsum, sume)
                    mask_nc = small.tile([GRP, C], f32, tag="masknc")
                    nc.vector.tensor_scalar_mul(out=mask_nc, in0=expm, scalar1=rsum)

                    # mask back to c-layout [P, NCH, GRP]
                    mt_ps = psB.tile([P, NCH, GRP], f32, tag="maskt")
                    for j in range(NCH):
                        nc.tensor.transpose(mt_ps[:, j, :], mask_nc[:, j * P : (j + 1) * P], ident_sb[:GRP, :GRP])
                    mask_cn = halfp.tile([P, NCH, GRP], f32, tag="maskcn")
                    nc.vector.tensor_copy(out=mask_cn, in_=mt_ps)

                    # masked = feat * mask (in place), store on the ACT HWDGE ring
                    for i in range(GRP):
                        t = img_tiles[i]
                        for j in range(NCH):
                            nc.vector.tensor_scalar_mul(
                                out=t[:, j, :], in0=t[:, j, :],
                                scalar1=mask_cn[:, j, i : i + 1],
                            )
                        nc.scalar.dma_start(out=masked_r[n0 + i], in_=t)

                    # pred = (sums @ w.T + HW*b) / HW
                    pr_ps = psB.tile([GRP, NCLS], f32, tag="pred")
                    for c0, cn in ((0, 512), (512, NCLS - 512)):
                        for j in range(NCH):
                            nc.tensor.matmul(
                                pr_ps[:, c0 : c0 + cn], lhsT=pooled[:, j, :],
                                rhs=w_t[:, j, c0 : c0 + cn],
                                start=(j == 0), stop=False,
                            )
                        nc.tensor.matmul(
                            pr_ps[:, c0 : c0 + cn], lhsT=ones_row[:, n0 : n0 + GRP],
                            rhs=b_sb[:, c0 : c0 + cn], start=False, stop=True,
                        )
                    pred_h = small.tile([GRP, NCLS], f32, tag="predsb")
                    nc.scalar.mul(out=pred_h, in_=pr_ps, mul=1.0 / HW)
                    nc.sync.dma_start(out=pred[n0 : n0 + GRP, :], in_=pred_h)

    _split_sync_waits(nc)
    return nc


def make_in_maps(feat, y, w, b):
    """Shard full inputs into per-core input dicts (host-side marshaling only)."""
    feat = np.ascontiguousarray(np.asarray(feat, dtype=np.float32))
    y = np.asarray(y).astype(np.float32)          # class ids < 1000: exact in f32
    w = np.ascontiguousarray(np.asarray(w, dtype=np.float32))
    b = np.asarray(b, dtype=np.float32)

    ident = np.eye(P, dtype=np.float32)
    iota = (np.arange(P, dtype=np.float32)[:, None]
            + P * np.arange(KCH, dtype=np.float32)[None, :]).astype(np.float32)
    brow = (b * float(HW)).reshape(1, NCLS)

    in_maps = []
    for i in range(NCORES):
        sl = slice(i * NPC, (i + 1) * NPC)
        in_maps.append({
            "feat": feat[sl].reshape(NPC, C, HW),
            "yrow": y[sl].reshape(1, NPC),
            "w": w,
            "brow": brow,
            "ident": ident,
            "iota8": iota,
        })
    return in_maps


_CACHE = {}


def _get_nc(repeat=1):
    if repeat not in _CACHE:
        _CACHE[repeat] = build_bass(repeat)
    return _CACHE[repeat]


def run_on_cores(in_maps, repeat=1, **kwargs):
    from concourse.bass_utils import run_bass_kernel_spmd

    nc = _get_nc(repeat)
    return run_bass_kernel_spmd(nc, in_maps, core_ids=list(range(NCORES)), **kwargs)


def kernel(feat, y, w, b):
    res = run_on_cores(make_in_maps(feat, y, w, b))
    masked = np.concatenate(
        [r["masked"].reshape(NPC, C, H, W) for r in res.results], axis=0
    )
    pred = np.concatenate([r["pred"] for r in res.results], axis=0)
    return masked.astype(np.float32), pred.astype(np.float32)


# revision 7
# speedup vs baseline: 42.5752x; 42.5752x over previous
"""Trainium2 Bass kernel for CSAFR probe+mask module.

Computation (per batch element n):
    pooled[n,c]  = mean_hw feat[n,c,:,:]
    pred[n,:]    = pooled[n] @ w.T + b
    g[n,c]       = w[y_n, c] * pooled[n,c]
    mask[n,:]    = softmax(1 + g[n]/||g[n]|| * sqrt(C)/2)
    masked[n]    = feat[n] * mask[n,c]

Sharding: data-parallel over batch N=128 -> 16 images per core x 8 cores.
w/b replicated. No collectives.
"""

import math

import numpy as np

# Problem constants (hardcoded per harness contract).
N, C, H, W = 128, 512, 28, 28
HW = H * W                       # 784
NCLS = 1000
NCORES = 8
NPC = N // NCORES                # 16 images per core
GRP = 8                          # images per half-batch
P = 128
NCH = C // P                     # 4 channel chunks
KCH = (NCLS + P - 1) // P        # 8 class chunks (last partial: 104 rows)
LASTK = NCLS - (KCH - 1) * P     # 104


def _split_sync_waits(nc, max_waits=1):
    """This walrus build's codegen rejects instructions carrying >max_waits sem
    waits. Hoist the overflow onto preceding same-engine NoOp carriers."""
    import concourse.mybir as mybir

    uid = [0]
    for fn in nc.m.functions:
        for bb in fn.blocks:
            insts = bb.instructions
            i = 0
            while i < len(insts):
                inst = insts[i]
                si = inst.sync_info
                if si is not None and si.on_wait and len(si.on_wait) > max_waits:
                    waits = list(si.on_wait)
                    extra = [
                        waits[j : j + max_waits]
                        for j in range(max_waits, len(waits), max_waits)
                    ]
                    si.on_wait = waits[:max_waits]
                    for chunk in extra:
                        uid[0] += 1
                        nd = mybir.InstNoOp(
                            name=f"SWsplit-{uid[0]}",
                            engine=inst.engine,
                            bass_nofuse=True,
                            sync_info=mybir.SyncInfo(on_wait=chunk, on_update=[]),
                        )
                        nc.register_instruction(nd)
                        insts.insert(i, nd)
                        i += 1
                i += 1
    return nc


def build_bass(repeat=1):
    """Build the per-core Bass module. `repeat` re-runs the body (same data,
    same result) for overhead-cancelling wall-clock timing in test harnesses."""
    import concourse.bass as bass
    import concourse.tile as tile
    from concourse import mybir

    f32 = mybir.dt.float32
    nc = bass.Bass()

    feat = nc.dram_tensor("feat", [NPC, C, HW], f32, kind="ExternalInput")
    yrow = nc.dram_tensor("yrow", [1, NPC], f32, kind="ExternalInput")
    w_in = nc.dram_tensor("w", [NCLS, C], f32, kind="ExternalInput")
    brow = nc.dram_tensor("brow", [1, NCLS], f32, kind="ExternalInput")  # pre-scaled by HW
    ident = nc.dram_tensor("ident", [P, P], f32, kind="ExternalInput")
    iota8 = nc.dram_tensor("iota8", [P, KCH], f32, kind="ExternalInput")
    masked = nc.dram_tensor("masked", [NPC, C, HW], f32, kind="ExternalOutput")
    pred = nc.dram_tensor("pred", [NPC, NCLS], f32, kind="ExternalOutput")

    # Per-image view: partition p <- channel j*128+p, free (j, hw).
    feat_r = feat[:, :, :].rearrange("n (j p) x -> n p j x", p=P)
    masked_r = masked[:, :, :].rearrange("n (j p) x -> n p j x", p=P)

    X = mybir.AxisListType.X
    EQ = mybir.AluOpType.is_equal
    MUL = mybir.AluOpType.mult

    with tile.TileContext(nc) as tc:
        with (
            tc.tile_pool(name="consts", bufs=1) as consts,
            tc.tile_pool(name="halfp", bufs=2) as halfp,
            tc.tile_pool(name="imgs", bufs=10) as imgs,
            tc.tile_pool(name="small", bufs=2) as small,
            tc.tile_pool(name="psA", bufs=3, space="PSUM") as psA,
            tc.tile_pool(name="psB", bufs=1, space="PSUM") as psB,
        ):
            # ---- constants (loaded once) ----
            ident_sb = consts.tile([P, P], f32)
            nc.sync.dma_start(out=ident_sb, in_=ident[:, :])
            iota_sb = consts.tile([P, KCH], f32)
            nc.sync.dma_start(out=iota_sb, in_=iota8[:, :])
            b_sb = consts.tile([1, NCLS], f32)
            nc.sync.dma_start(out=b_sb, in_=brow[:, :])
            ones_row = consts.tile([1, NPC], f32)
            nc.vector.memset(ones_row, 1.0)
            ones_col = consts.tile([P, 1], f32)
            nc.vector.memset(ones_col, 1.0)

            # y broadcast down all partitions (partition-stride-0 DMA).
            y_bc = consts.tile([P, NPC], f32)
            y_b_ap = bass.AP(tensor=yrow[:, :].tensor, offset=0, ap=[[0, P], [1, NPC]])
            nc.gpsimd.dma_start(out=y_bc, in_=y_b_ap)

            # w in [cls_chunk-on-partition] layout: w_sb[q, k*C+c] = w[k*128+q, c]
            w_sb = consts.tile([P, KCH * C], f32)
            w_head = w_in[: (KCH - 1) * P, :].rearrange("(k q) c -> q k c", q=P)
            nc.sync.dma_start(out=w_sb[:, : (KCH - 1) * C].rearrange("q (k c) -> q k c", k=KCH - 1), in_=w_head)
            # zero the whole tail chunk first (engine-op partition starts must be
            # 0/32/64/96), then overwrite rows 0..LASTK-1 with the real data
            nc.vector.memset(w_sb[:, (KCH - 1) * C :], 0.0)
            nc.sync.dma_start(out=w_sb[:LASTK, (KCH - 1) * C :], in_=w_in[(KCH - 1) * P :, :])

            # w transposed to [c-on-partition, cls] via PE: w_t[p, j, cls] = w[cls, j*128+p]
            w_t = consts.tile([P, NCH, NCLS], f32)
            for k in range(KCH):
                ncls_k = P if k < KCH - 1 else LASTK
                for j in range(NCH):
                    tp = psA.tile([P, P], f32, tag="wt")
                    nc.tensor.transpose(tp, w_sb[:, k * C + j * P : k * C + (j + 1) * P], ident_sb)
                    nc.scalar.copy(out=w_t[:, j, k * P : k * P + ncls_k], in_=tp[:, :ncls_k])

            # one-hot of y in cls-chunk layout: oh[p, k, n] = (y_n == k*128+p)
            oh = consts.tile([P, KCH, NPC], f32)
            for k in range(KCH):
                nc.vector.tensor_scalar(
                    out=oh[:, k, :], in0=y_bc, scalar1=iota_sb[:, k : k + 1],
                    scalar2=None, op0=EQ,
                )

            for _rep in range(repeat):
                for h in range(2):  # half-batches of GRP images
                    n0 = h * GRP
                    # pooled sums in c-layout: pooled[p, j, i] = sum_hw feat[n0+i, j*128+p, :]
                    pooled = halfp.tile([P, NCH, GRP], f32, tag="pooled")
                    img_tiles = []
                    for i in range(GRP):
                        t = imgs.tile([P, NCH, HW], f32, tag="img")
                        nc.sync.dma_start(out=t, in_=feat_r[n0 + i])
                        nc.vector.reduce_sum(out=pooled[:, :, i], in_=t, axis=X)
                        img_tiles.append(t)

                    # pooled -> n-layout [GRP, C] (PE transpose, then copy off PSUM)
                    pnc_ps = psB.tile([GRP, C], f32, tag="pooledt")
                    for j in range(NCH):
                        nc.tensor.transpose(pnc_ps[:, j * P : (j + 1) * P], pooled[:, j, :], ident_sb)
                    pnc = small.tile([GRP, C], f32, tag="pnc")
                    nc.scalar.copy(out=pnc, in_=pnc_ps)

                    # w[y] gather as one-hot matmul: wsel[i, c] = sum_cls oh[cls, i] * w[cls, c]
                    ws_ps = psB.tile([GRP, C], f32, tag="wsel")
                    for k in range(KCH):
                        nc.tensor.matmul(
                            ws_ps, lhsT=oh[:, k, n0 : n0 + GRP],
                            rhs=w_sb[:, k * C : (k + 1) * C],
                            start=(k == 0), stop=(k == KCH - 1),
                        )

                    # g = wsel * pooled_sums  (scale-invariant for the mask: g/||g||)
                    g = small.tile([GRP, C], f32, tag="g")
                    nc.vector.tensor_mul(g, ws_ps, pnc)

                    # ||g||: sum of squares via ACT accumulate, sqrt, reciprocal
                    gsq = small.tile([GRP, C], f32, tag="gsq")
                    ssq = small.tile([GRP, 1], f32, tag="ssq")
                    nc.scalar.activation(out=gsq, in_=g, func=mybir.ActivationFunctionType.Square, accum_out=ssq)
                    nrm = small.tile([GRP, 1], f32, tag="nrm")
                    nc.scalar.sqrt(nrm, ssq)
                    rnrm = small.tile([GRP, 1], f32, tag="rnrm")
                    nc.vector.reciprocal(rnrm, nrm)
                    scl = small.tile([GRP, 1], f32, tag="scl")
                    nc.scalar.mul(scl, rnrm, math.sqrt(C) / 2.0)

                    # softmax without max-subtraction (args bounded in 1 +- sqrt(C)/2):
                    # expm = exp(g*scl + 1), sume = row sum (fused accum)
                    expm = small.tile([GRP, C], f32, tag="expm")
                    sume = small.tile([GRP, 1], f32, tag="sume")
                    nc.scalar.activation(
                        out=expm, in_=g, func=mybir.ActivationFunctionType.Exp,
                        bias=ones_col[:GRP], scale=scl, accum_out=sume,
                    )
                    rsum = small.tile([GRP, 1], f32, tag="rsum")
                    nc.vector.reciprocal(rsum, sume)
                    mask_nc = small.tile([GRP, C], f32, tag="masknc")
                    nc.vector.tensor_scalar_mul(out=mask_nc, in0=expm, scalar1=rsum)

                    # mask back to c-layout [P, NCH, GRP]
                    mt_ps = psB.tile([P, NCH, GRP], f32, tag="maskt")
                    for j in range(NCH):
                        nc.tensor.transpose(mt_ps[:, j, :], mask_nc[:, j * P : (j + 1) * P], ident_sb[:GRP, :GRP])
                    mask_cn = halfp.tile([P, NCH, GRP], f32, tag="maskcn")
                    nc.vector.tensor_copy(out=mask_cn, in_=mt_ps)

                    # masked = feat * mask (in place), store on the ACT HWDGE ring
                    for i in range(GRP):
                        t = img_tiles[i]
                        for j in range(NCH):
                            nc.vector.tensor_scalar_mul(
                                out=t[:, j, :], in0=t[:, j, :],
                                scalar1=mask_cn[:, j, i : i + 1],
                            )
                        nc.scalar.dma_start(out=masked_r[n0 + i], in_=t)

                    # pred = (sums @ w.T + HW*b) / HW
                    pr_ps = psB.tile([GRP, NCLS], f32, tag="pred")
                    for c0, cn in ((0, 512), (512, NCLS - 512)):
                        for j in range(NCH):
                            nc.tensor.matmul(
                                pr_ps[:, c0 : c0 + cn], lhsT=pooled[:, j, :],
                                rhs=w_t[:, j, c0 : c0 + cn],
                                start=(j == 0), stop=False,
                            )
                        nc.tensor.matmul(
                            pr_ps[:, c0 : c0 + cn], lhsT=ones_row[:, n0 : n0 + GRP],
                            rhs=b_sb[:, c0 : c0 + cn], start=False, stop=True,
                        )
                    pred_h = small.tile([GRP, NCLS], f32, tag="predsb")
                    nc.scalar.mul(out=pred_h, in_=pr_ps, mul=1.0 / HW)
                    nc.sync.dma_start(out=pred[n0 : n0 + GRP, :], in_=pred_h)

    _split_sync_waits(nc)
    return nc


def make_in_maps(feat, y, w, b):
    """Shard full inputs into per-core input dicts (host-side marshaling only)."""
    feat = np.ascontiguousarray(np.asarray(feat, dtype=np.float32))
    y = np.asarray(y).astype(np.float32)          # class ids < 1000: exact in f32
    w = np.ascontiguousarray(np.asarray(w, dtype=np.float32))
    b = np.asarray(b, dtype=np.float32)

    ident = np.eye(P, dtype=np.float32)
    iota = (np.arange(P, dtype=np.float32)[:, None]
            + P * np.arange(KCH, dtype=np.float32)[None, :]).astype(np.float32)
    brow = (b * float(HW)).reshape(1, NCLS)

    in_maps = []
    for i in range(NCORES):
        sl = slice(i * NPC, (i + 1) * NPC)
        in_maps.append({
            "feat": feat[sl].reshape(NPC, C, HW),
            "yrow": y[sl].reshape(1, NPC),
            "w": w,
            "brow": brow,
            "ident": ident,
            "iota8": iota,
        })
    return in_maps


_CACHE = {}


def _get_nc(repeat=1):
    if repeat not in _CACHE:
        _CACHE[repeat] = build_bass(repeat)
    return _CACHE[repeat]


def run_on_cores(in_maps, repeat=1, **kwargs):
    from concourse.bass_utils import run_bass_kernel_spmd

    nc = _get_nc(repeat)
    return run_bass_kernel_spmd(nc, in_maps, core_ids=list(range(NCORES)), **kwargs)


def make_runner(repeat=1):
    """Jitted 8-core runner mirroring bass2jax.run_bass_via_pjrt, but reusable
    with device-resident inputs (for overhead-free timing loops).

    Returns (run, put) where put(in_maps) -> device args and run(*args) ->
    tuple of concatenated outputs [masked (128,512,784), pred (128,1000)].
    """
    import jax
    from jax.experimental.shard_map import shard_map
    from jax.sharding import Mesh, NamedSharding, PartitionSpec

    from concourse import bass2jax, mybir

    bass2jax.install_neuronx_cc_hook()
    nc = _get_nc(repeat)
    partition_name = nc.partition_id_tensor.name if nc.partition_id_tensor else None

    in_names, out_names, out_avals, zero_outs = [], [], [], []
    for alloc in nc.m.functions[0].allocations:
        if not isinstance(alloc, mybir.MemoryLocationSet):
            continue
        name = alloc.memorylocations[0].name
        if alloc.kind == "ExternalInput":
            if name != partition_name:
                in_names.append(name)
        elif alloc.kind == "ExternalOutput":
            out_names.append(name)
            shape = tuple(alloc.tensor_shape)
            dtype = mybir.dt.np(alloc.dtype)
            out_avals.append(jax.core.ShapedArray(shape, dtype))
            zero_outs.append(np.zeros(shape, dtype))
    n_params = len(in_names)
    param_names = list(in_names)
    all_in = in_names + out_names + ([partition_name] if partition_name else [])

    def _body(*args):
        operands = list(args)
        if partition_name:
            operands.append(bass2jax.partition_id_tensor())
        outs = bass2jax._bass_exec_p.bind(
            *operands,
            out_avals=tuple(out_avals),
            in_names=tuple(all_in),
            out_names=tuple(out_names),
            lowering_input_output_aliases=(),
            sim_require_finite=True,
            sim_require_nnan=True,
            nc=nc,
        )
        return tuple(outs)

    devices = jax.devices()[:NCORES]
    mesh = Mesh(np.asarray(devices), ("core",))
    n_outs = len(out_names)
    in_specs = (PartitionSpec("core"),) * (n_params + n_outs)
    out_specs = (PartitionSpec("core"),) * n_outs
    run = jax.jit(
        shard_map(_body, mesh=mesh, in_specs=in_specs, out_specs=out_specs,
                  check_rep=False),
        keep_unused=True,
    )
    sh = NamedSharding(mesh, PartitionSpec("core"))

    def put(in_maps):
        args = []
        for name in param_names:
            cat = np.concatenate([np.asarray(m[name]) for m in in_maps], axis=0)
            args.append(jax.device_put(cat, sh))
        for z in zero_outs:
            cat = np.zeros((NCORES * z.shape[0], *z.shape[1:]), z.dtype)
            args.append(jax.device_put(cat, sh))
        return args

    return run, put


def kernel(feat, y, w, b):
    res = run_on_cores(make_in_maps(feat, y, w, b))
    masked = np.concatenate(
        [r["masked"].reshape(NPC, C, H, W) for r in res.results], axis=0
    )
    pred = np.concatenate([r["pred"] for r in res.results], axis=0)
    return masked.astype(np.float32), pred.astype(np.float32)
